# revision 9
# baseline (speedup 1.0000x reference)
"""Trainium2 Bass kernel for nn_KerasCustomMappingLayer (osu-style map construction).

Strategy (pure data-parallel over 8 NeuronCores, B=1048576 rows):
  - Each core handles B/8 = 131072 rows laid out as 128 partitions x 1024
    elements; processed in ~7 tiles of F~147 elements per partition.
  - All 10 per-step scalars are host-known at build time -> each scan step is
    specialized on (rerand, is_slider). Positions are kept in the scaled
    domain x/XMAX, y/YMAX so outputs c0/c1/c4/c5 need no extra division.
  - rsqrt(c^2+s^2) via custom DVE hypot2 + ACT Exp(-0.5*Ln(x)) (Rsqrt banned).
  - The wall-clamp update is ONE custom DVE op per axis:
      _x = px + dx + 2*((px<wl)*relu(-dx) - (px>wr)*relu(dx))
  - Slider outputs via a lincomb custom op: out = a*C0 + b*C1.
  - Circle outputs are ACT copies; engine balance: DVE customs + 2 NT muls,
    ACT ln/exp/dx/copies, GPSIMD 2 NT muls.
"""
import sys
import numpy as np

for _p in ("/opt/trn_rl_repo",):
    if _p not in sys.path:
        sys.path.insert(0, _p)

NGS = 10
XMAX, YMAX = 512.0, 384.0
LMUL, MTFD = 1.0, 1.0
N_CORES = 8
P = 128

_OPS = {}
_NC_CACHE = {}


def _get_custom_ops():
    global _OPS
    if _OPS:
        return _OPS
    import concourse.dve_ops as dve_ops
    from concourse.dve_spec import Spec, Src0, Src1, C0, C1, C2, relu, sq
    from concourse.dve_uop import DveOpSpec

    defs = {
        "ANT_HYPOT2": dict(
            body=sq(Src0) + sq(Src1),
            reference=lambda in0, in1, s0, s1, imm2: (
                in0.astype(np.float32) ** 2 + in1.astype(np.float32) ** 2),
        ),
        # t1 = px + (px<wl)*RN2 - 0.5*RN2        (RN2 = relu(-2dx))
        "ANT_WALLQ1": dict(
            body=Src0 + (Src0 < C0) * Src1 - Src1 * C1,
            reference=lambda in0, in1, s0, s1, imm2: (
                in0.astype(np.float32) + (in0 < s0) * in1 - in1 * np.float32(s1)),
        ),
        # _x = t1 + 0.5*RP2 - (t1>wr)*RP2        (RP2 = relu(+2dx))
        "ANT_WALLQ2": dict(
            body=Src0 + Src1 * C1 - (Src0 > C0) * Src1,
            reference=lambda in0, in1, s0, s1, imm2: (
                in0.astype(np.float32) + in1 * np.float32(s1) - (in0 > s0) * in1),
        ),
        "ANT_LINCOMB": dict(
            body=Src0 * C0 + Src1 * C1,
            reference=lambda in0, in1, s0, s1, imm2: (
                in0.astype(np.float32) * s0 + in1.astype(np.float32) * s1),
        ),
    }
    ops = {}
    for name, d in defs.items():
        existing = next((o for o in dve_ops.OPS if o.name == name), None)
        if existing is not None:
            ops[name] = existing
            continue
        spec = Spec(body=d["body"], reference=d["reference"])
        row = max(dve_ops._SUB_OPCODE_FOR_NAME.values()) + 1
        assert row < 0x20, "custom DVE row overflow"
        dve_ops._SUB_OPCODE_FOR_NAME[name] = row
        shas = {}
        for ver in ("v3", "v4"):
            try:
                uops = dve_ops.lower(spec, ver=ver)
                shas[ver] = DveOpSpec(
                    name=name, opcode=row, uops=uops,
                    rd1_en=dve_ops.has_src1(spec)).sha(ver)
            except Exception:
                pass
        assert shas, f"lower() failed for {name}"
        op = dve_ops.DveOp(name, spec, subdim=False, uops_sha=shas)
        dve_ops.OPS.append(op)
        dve_ops.CUSTOM_DVE_SPECS[name] = spec
        ops[name] = op
    _OPS = ops
    return ops


def _host_consts(slider_lengths, slider_cos_each, slider_sin_each,
                 note_distances, tick_diff, start_pos, is_slider):
    f = np.float32
    l = (f(LMUL) * note_distances.astype(f)).astype(f)
    return dict(
        wl=tuple(float(x) for x in (f(0.05 * XMAX) + l * f(0.5)) / f(XMAX)),
        wr=tuple(float(x) for x in (f(0.95 * XMAX) - l * f(0.5)) / f(XMAX)),
        wt=tuple(float(x) for x in (f(0.05 * YMAX) + l * f(0.5)) / f(YMAX)),
        wb=tuple(float(x) for x in (f(0.95 * YMAX) - l * f(0.5)) / f(YMAX)),
        lkx=tuple(float(x) for x in l / f(XMAX)),
        lky=tuple(float(x) for x in l / f(YMAX)),
        rr=tuple(int(x) for x in (tick_diff.astype(f) > f(MTFD))),
        isl=tuple(int(x) for x in (np.asarray(is_slider) != 0)),
        slnx=tuple(float(x) for x in slider_lengths.astype(f) / f(XMAX)),
        slny=tuple(float(x) for x in slider_lengths.astype(f) / f(YMAX)),
        scos=tuple(float(x) for x in slider_cos_each.astype(f)),
        ssin=tuple(float(x) for x in slider_sin_each.astype(f)),
        px0=float(f(start_pos[0]) / f(XMAX)),
        py0=float(f(start_pos[1]) / f(YMAX)),
    )


def _build(c, b_core, n_tiles=7):
    import concourse.bacc as bacc
    import concourse.mybir as mybir
    from concourse.tile import TileContext

    f32 = mybir.dt.float32
    AF = mybir.ActivationFunctionType
    ops = _get_custom_ops()
    HYP, LIN = ops["ANT_HYPOT2"], ops["ANT_LINCOMB"]
    Q1, Q2 = ops["ANT_WALLQ1"], ops["ANT_WALLQ2"]

    npp = b_core // P                       # elements per partition (1024)
    base, rem = divmod(npp, n_tiles)
    Fs = [base + (1 if t < rem else 0) for t in range(n_tiles)]

    nc = bacc.Bacc("TRN2", target_bir_lowering=False, debug=False)
    var = nc.dram_tensor("var", [b_core, 2 * NGS * 2], f32, kind="ExternalInput")
    out = nc.dram_tensor("out", [b_core, NGS * 6], f32, kind="ExternalOutput")
    varv = var.rearrange("(p n) c -> p n c", p=P)
    outv = out.rearrange("(p n) c -> p n c", p=P)

    with TileContext(nc) as tc:
        with tc.tile_pool(name="io", bufs=2) as iop, \
             tc.tile_pool(name="work", bufs=2) as wp, \
             tc.tile_pool(name="ph0", bufs=1) as ph0p, \
             tc.tile_pool(name="cst", bufs=1) as cp:
            Fmax = max(Fs)
            px0t = cp.tile([P, Fmax], f32, tag="px0")
            py0t = cp.tile([P, Fmax], f32, tag="py0")
            nc.vector.memset(px0t[:], c["px0"])
            nc.vector.memset(py0t[:], c["py0"])
            czero = cp.tile([P, 1], f32, tag="czero")
            chalf = cp.tile([P, 1], f32, tag="chalf")
            nc.vector.memset(czero[:], 0.0)
            nc.vector.memset(chalf[:], 0.5)
            nc.const_aps.aps[(f32, 0.0)] = czero[:]
            nc.const_aps.aps[(f32, 0.5)] = chalf[:]
            off = 0
            for F in Fs:
                tin = iop.tile([P, F, 4 * NGS], f32, tag="tin")
                nc.sync.dma_start(tin[:], varv[:, off:off + F, :])
                tout = iop.tile([P, F, 6 * NGS], f32, tag="tout")
                nt = wp.tile([P, F, 4 * NGS], f32, tag="nt")
                ssum = ph0p.tile([P, F, 2 * NGS], f32, tag="ssum")
                rn = ph0p.tile([P, F, 2 * NGS], f32, tag="rn")

                # ---- phase 0: rn = (c^2+s^2)^-0.5 ; nt = normalized dirs ----
                nc.vector._custom_dve(HYP, out=ssum[:], in0=tin[:, :, 0:20],
                                      in1=tin[:, :, 20:40])
                nc.scalar.activation(rn[:], ssum[:], AF.Ln)
                nc.scalar.activation(rn[:], rn[:], AF.Exp, scale=-0.5)
                # nt per element: [c0 s0 c1 s1 .. c9 s9 | ch0 sh0 .. ch9 sh9]
                nc.vector.tensor_mul(nt[:, :, 0:20:2], tin[:, :, 0:10], rn[:, :, 0:10])
                nc.vector.tensor_mul(nt[:, :, 1:21:2], tin[:, :, 20:30], rn[:, :, 0:10])
                nc.gpsimd.tensor_mul(nt[:, :, 20:40:2], tin[:, :, 10:20], rn[:, :, 10:20])
                nc.gpsimd.tensor_mul(nt[:, :, 21:40:2], tin[:, :, 30:40], rn[:, :, 10:20])

                pxs, pys = px0t[:, 0:F], py0t[:, 0:F]
                for k in range(NGS):
                    c0 = tout[:, :, 6 * k]
                    c1 = tout[:, :, 6 * k + 1]
                    if c["rr"][k]:
                        # _x' = 0.5*vk + 0.5 ; _y' = 0.5*vk2 + 0.5  (one op, pair AP)
                        nc.scalar.activation(tout[:, :, 6 * k:6 * k + 2],
                                             tin[:, :, k:k + 21:20],
                                             AF.Identity, bias=0.5, scale=0.5)
                    else:
                        rn2x = wp.tile([P, F], f32, tag="rn2x")
                        rp2x = wp.tile([P, F], f32, tag="rp2x")
                        rn2y = wp.tile([P, F], f32, tag="rn2y")
                        rp2y = wp.tile([P, F], f32, tag="rp2y")
                        nc.scalar.activation(rn2x[:], nt[:, :, 2 * k], AF.Relu,
                                             scale=-2.0 * c["lkx"][k])
                        nc.scalar.activation(rp2x[:], nt[:, :, 2 * k], AF.Relu,
                                             scale=2.0 * c["lkx"][k])
                        nc.scalar.activation(rn2y[:], nt[:, :, 2 * k + 1], AF.Relu,
                                             scale=-2.0 * c["lky"][k])
                        nc.scalar.activation(rp2y[:], nt[:, :, 2 * k + 1], AF.Relu,
                                             scale=2.0 * c["lky"][k])
                        nc.vector._custom_dve(Q1, out=c0, in0=pxs, in1=rn2x[:],
                                              s0=c["wl"][k], s1=0.5)
                        nc.vector._custom_dve(Q2, out=c0, in0=c0, in1=rp2x[:],
                                              s0=c["wr"][k], s1=0.5)
                        nc.vector._custom_dve(Q1, out=c1, in0=pys, in1=rn2y[:],
                                              s0=c["wt"][k], s1=0.5)
                        nc.vector._custom_dve(Q2, out=c1, in0=c1, in1=rp2y[:],
                                              s0=c["wb"][k], s1=0.5)
                    if c["isl"][k]:
                        ch = nt[:, :, 20 + 2 * k]
                        sh = nt[:, :, 21 + 2 * k]
                        nc.vector._custom_dve(LIN, out=tout[:, :, 6 * k + 2], in0=ch,
                                              in1=sh, s0=c["scos"][k], s1=-c["ssin"][k])
                        nc.vector._custom_dve(LIN, out=tout[:, :, 6 * k + 3], in0=ch,
                                              in1=sh, s0=c["ssin"][k], s1=c["scos"][k])
                        nc.vector._custom_dve(LIN, out=tout[:, :, 6 * k + 4], in0=c0,
                                              in1=ch, s0=1.0, s1=c["slnx"][k])
                        nc.vector._custom_dve(LIN, out=tout[:, :, 6 * k + 5], in0=c1,
                                              in1=sh, s0=1.0, s1=c["slny"][k])
                    else:
                        j0 = 20 + 2 * k if c["rr"][k] else 2 * k
                        nc.scalar.copy(tout[:, :, 6 * k + 2:6 * k + 4],
                                       nt[:, :, j0:j0 + 2])
                        nc.scalar.copy(tout[:, :, 6 * k + 4:6 * k + 6],
                                       tout[:, :, 6 * k:6 * k + 2])
                    pxs, pys = c0, c1
                nc.sync.dma_start(outv[:, off:off + F, :], tout[:])
                off += F
    nc.compile()
    return nc


def kernel(**inputs):
    var = np.ascontiguousarray(np.asarray(inputs["var_tensor"], dtype=np.float32))
    B = var.shape[0]
    assert B % (N_CORES * P) == 0
    b_core = B // N_CORES
    c = _host_consts(
        np.asarray(inputs["slider_lengths"]), np.asarray(inputs["slider_cos_each"]),
        np.asarray(inputs["slider_sin_each"]), np.asarray(inputs["note_distances"]),
        np.asarray(inputs["tick_diff"]), np.asarray(inputs["start_pos"]),
        np.asarray(inputs["is_slider"]))
    key = (B, tuple(sorted(c.items())))
    if key not in _NC_CACHE:
        _NC_CACHE[key] = _build(c, b_core)
    nc = _NC_CACHE[key]

    from concourse.bass_utils import run_bass_kernel_spmd
    in_maps = [{"var": var[i * b_core:(i + 1) * b_core]} for i in range(N_CORES)]
    res = run_bass_kernel_spmd(nc, in_maps, core_ids=list(range(N_CORES)))
    out = np.concatenate([r["out"] for r in res.results], axis=0)
    return out.reshape(B, NGS, 6)


# revision 13
# speedup vs baseline: 1.0153x; 1.0153x over previous
"""Trainium2 Bass kernel for nn_KerasCustomMappingLayer (osu-style map construction).

Strategy (pure data-parallel over 8 NeuronCores, B=1048576 rows):
  - Each core handles B/8 = 131072 rows laid out as 128 partitions x 1024
    elements; processed in ~7 tiles of F~147 elements per partition.
  - All 10 per-step scalars are host-known at build time -> each scan step is
    specialized on (rerand, is_slider). Positions are kept in the scaled
    domain x/XMAX, y/YMAX so outputs c0/c1/c4/c5 need no extra division.
  - rsqrt(c^2+s^2) via custom DVE hypot2 + ACT Exp(-0.5*Ln(x)) (Rsqrt banned).
  - The wall-clamp update is ONE custom DVE op per axis:
      _x = px + dx + 2*((px<wl)*relu(-dx) - (px>wr)*relu(dx))
  - Slider outputs via a lincomb custom op: out = a*C0 + b*C1.
  - Circle outputs are ACT copies; engine balance: DVE customs + 2 NT muls,
    ACT ln/exp/dx/copies, GPSIMD 2 NT muls.
"""
import sys
import numpy as np

for _p in ("/opt/trn_rl_repo",):
    if _p not in sys.path:
        sys.path.insert(0, _p)

NGS = 10
XMAX, YMAX = 512.0, 384.0
LMUL, MTFD = 1.0, 1.0
N_CORES = 8
P = 128

_OPS = {}
_NC_CACHE = {}


def _get_custom_ops():
    global _OPS
    if _OPS:
        return _OPS
    import concourse.dve_ops as dve_ops
    from concourse.dve_spec import Spec, Src0, Src1, C0, C1, C2, relu, sq
    from concourse.dve_uop import DveOpSpec

    defs = {
        "ANT_HYPOT2": dict(
            body=sq(Src0) + sq(Src1),
            reference=lambda in0, in1, s0, s1, imm2: (
                in0.astype(np.float32) ** 2 + in1.astype(np.float32) ** 2),
        ),
        # t1 = px + (px<wl)*RN2 - 0.5*RN2        (RN2 = relu(-2dx))
        "ANT_WALLQ1": dict(
            body=Src0 + (Src0 < C0) * Src1 - Src1 * C1,
            reference=lambda in0, in1, s0, s1, imm2: (
                in0.astype(np.float32) + (in0 < s0) * in1 - in1 * np.float32(s1)),
        ),
        # _x = t1 + 0.5*RP2 - (t1>wr)*RP2        (RP2 = relu(+2dx))
        "ANT_WALLQ2": dict(
            body=Src0 + Src1 * C1 - (Src0 > C0) * Src1,
            reference=lambda in0, in1, s0, s1, imm2: (
                in0.astype(np.float32) + in1 * np.float32(s1) - (in0 > s0) * in1),
        ),
        "ANT_LINCOMB": dict(
            body=Src0 * C0 + Src1 * C1,
            reference=lambda in0, in1, s0, s1, imm2: (
                in0.astype(np.float32) * s0 + in1.astype(np.float32) * s1),
        ),
    }
    ops = {}
    for name, d in defs.items():
        existing = next((o for o in dve_ops.OPS if o.name == name), None)
        if existing is not None:
            ops[name] = existing
            continue
        spec = Spec(body=d["body"], reference=d["reference"])
        row = max(dve_ops._SUB_OPCODE_FOR_NAME.values()) + 1
        assert row < 0x20, "custom DVE row overflow"
        dve_ops._SUB_OPCODE_FOR_NAME[name] = row
        shas = {}
        for ver in ("v3", "v4"):
            try:
                uops = dve_ops.lower(spec, ver=ver)
                shas[ver] = DveOpSpec(
                    name=name, opcode=row, uops=uops,
                    rd1_en=dve_ops.has_src1(spec)).sha(ver)
            except Exception:
                pass
        assert shas, f"lower() failed for {name}"
        op = dve_ops.DveOp(name, spec, subdim=False, uops_sha=shas)
        dve_ops.OPS.append(op)
        dve_ops.CUSTOM_DVE_SPECS[name] = spec
        ops[name] = op
    _OPS = ops
    return ops


def _host_consts(slider_lengths, slider_cos_each, slider_sin_each,
                 note_distances, tick_diff, start_pos, is_slider):
    f = np.float32
    l = (f(LMUL) * note_distances.astype(f)).astype(f)
    return dict(
        wl=tuple(float(x) for x in (f(0.05 * XMAX) + l * f(0.5)) / f(XMAX)),
        wr=tuple(float(x) for x in (f(0.95 * XMAX) - l * f(0.5)) / f(XMAX)),
        wt=tuple(float(x) for x in (f(0.05 * YMAX) + l * f(0.5)) / f(YMAX)),
        wb=tuple(float(x) for x in (f(0.95 * YMAX) - l * f(0.5)) / f(YMAX)),
        lkx=tuple(float(x) for x in l / f(XMAX)),
        lky=tuple(float(x) for x in l / f(YMAX)),
        rr=tuple(int(x) for x in (tick_diff.astype(f) > f(MTFD))),
        isl=tuple(int(x) for x in (np.asarray(is_slider) != 0)),
        slnx=tuple(float(x) for x in slider_lengths.astype(f) / f(XMAX)),
        slny=tuple(float(x) for x in slider_lengths.astype(f) / f(YMAX)),
        scos=tuple(float(x) for x in slider_cos_each.astype(f)),
        ssin=tuple(float(x) for x in slider_sin_each.astype(f)),
        px0=float(f(start_pos[0]) / f(XMAX)),
        py0=float(f(start_pos[1]) / f(YMAX)),
    )


def _build(c, b_core, n_tiles=7):
    import concourse.bacc as bacc
    import concourse.mybir as mybir
    from concourse.tile import TileContext

    f32 = mybir.dt.float32
    AF = mybir.ActivationFunctionType
    ops = _get_custom_ops()
    HYP, LIN = ops["ANT_HYPOT2"], ops["ANT_LINCOMB"]
    Q1, Q2 = ops["ANT_WALLQ1"], ops["ANT_WALLQ2"]

    npp = b_core // P                       # elements per partition (1024)
    base, rem = divmod(npp, n_tiles)
    Fs = [base + (1 if t < rem else 0) for t in range(n_tiles)]

    # which normalized pairs j are consumed, given the specialized steps:
    #   wall step k (rr=0): NT pair k;  circle rr=0: NT pair k (covered)
    #   slider k or circle rr=1: NTH pair 10+k
    needed = {k for k in range(NGS) if not c["rr"][k]}
    needed |= {NGS + k for k in range(NGS) if c["isl"][k] or c["rr"][k]}
    j0, j1 = min(needed), max(needed) + 1          # contiguous cover window
    njl = max(0, min(j1, NGS) - j0)                # low-half pairs in window
    njh = max(0, j1 - max(j0, NGS))                # high-half pairs in window

    nc = bacc.Bacc("TRN2", target_bir_lowering=False, debug=False)
    var = nc.dram_tensor("var", [b_core, 2 * NGS * 2], f32, kind="ExternalInput")
    out = nc.dram_tensor("out", [b_core, NGS * 6], f32, kind="ExternalOutput")
    varv = var.rearrange("(p n) c -> p n c", p=P)
    outv = out.rearrange("(p n) c -> p n c", p=P)

    with TileContext(nc) as tc:
        with tc.tile_pool(name="io", bufs=2) as iop, \
             tc.tile_pool(name="work", bufs=2) as wp, \
             tc.tile_pool(name="ph0", bufs=1) as ph0p, \
             tc.tile_pool(name="cst", bufs=1) as cp:
            Fmax = max(Fs)
            px0t = cp.tile([P, Fmax], f32, tag="px0")
            py0t = cp.tile([P, Fmax], f32, tag="py0")
            nc.vector.memset(px0t[:], c["px0"])
            nc.vector.memset(py0t[:], c["py0"])
            czero = cp.tile([P, 1], f32, tag="czero")
            chalf = cp.tile([P, 1], f32, tag="chalf")
            nc.vector.memset(czero[:], 0.0)
            nc.vector.memset(chalf[:], 0.5)
            nc.const_aps.aps[(f32, 0.0)] = czero[:]
            nc.const_aps.aps[(f32, 0.5)] = chalf[:]
            off = 0
            for F in Fs:
                tin = iop.tile([P, F, 4 * NGS], f32, tag="tin")
                nc.sync.dma_start(tin[:], varv[:, off:off + F, :])
                tout = iop.tile([P, F, 6 * NGS], f32, tag="tout")
                nt = wp.tile([P, F, 4 * NGS], f32, tag="nt")
                nj = j1 - j0
                ssum = ph0p.tile([P, F, nj], f32, tag="ssum")
                rn = ph0p.tile([P, F, nj], f32, tag="rn")

                # ---- phase 0: rn = (c^2+s^2)^-0.5 over the needed j window ----
                nc.vector._custom_dve(HYP, out=ssum[:], in0=tin[:, :, j0:j1],
                                      in1=tin[:, :, 20 + j0:20 + j1])
                nc.scalar.activation(rn[:], ssum[:], AF.Ln)
                nc.scalar.activation(rn[:], rn[:], AF.Exp, scale=-0.5)
                # nt per element: [c0 s0 c1 s1 .. c9 s9 | ch0 sh0 .. ch9 sh9]
                if njl:
                    lo0, lo1 = j0, j0 + njl
                    nc.vector.tensor_mul(nt[:, :, 2 * lo0:2 * lo1:2],
                                         tin[:, :, lo0:lo1], rn[:, :, 0:njl])
                    nc.gpsimd.tensor_mul(nt[:, :, 2 * lo0 + 1:2 * lo1:2],
                                         tin[:, :, 20 + lo0:20 + lo1], rn[:, :, 0:njl])
                if njh:
                    hi0 = max(j0, NGS)
                    nc.vector.tensor_mul(nt[:, :, 2 * hi0:2 * j1:2],
                                         tin[:, :, hi0:j1], rn[:, :, nj - njh:nj])
                    nc.gpsimd.tensor_mul(nt[:, :, 2 * hi0 + 1:2 * j1:2],
                                         tin[:, :, 20 + hi0:20 + j1], rn[:, :, nj - njh:nj])

                pxs, pys = px0t[:, 0:F], py0t[:, 0:F]
                for k in range(NGS):
                    c0 = tout[:, :, 6 * k]
                    c1 = tout[:, :, 6 * k + 1]
                    if c["rr"][k]:
                        # _x' = 0.5*vk + 0.5 ; _y' = 0.5*vk2 + 0.5  (one op, pair AP)
                        nc.vector.tensor_scalar(tout[:, :, 6 * k:6 * k + 2],
                                                tin[:, :, k:k + 21:20],
                                                0.5, 0.5,
                                                mybir.AluOpType.mult,
                                                mybir.AluOpType.add)
                    else:
                        rn2x = wp.tile([P, F], f32, tag="rn2x")
                        rp2x = wp.tile([P, F], f32, tag="rp2x")
                        rn2y = wp.tile([P, F], f32, tag="rn2y")
                        rp2y = wp.tile([P, F], f32, tag="rp2y")
                        nc.scalar.activation(rn2x[:], nt[:, :, 2 * k], AF.Relu,
                                             scale=-2.0 * c["lkx"][k])
                        nc.scalar.activation(rp2x[:], nt[:, :, 2 * k], AF.Relu,
                                             scale=2.0 * c["lkx"][k])
                        nc.scalar.activation(rn2y[:], nt[:, :, 2 * k + 1], AF.Relu,
                                             scale=-2.0 * c["lky"][k])
                        nc.scalar.activation(rp2y[:], nt[:, :, 2 * k + 1], AF.Relu,
                                             scale=2.0 * c["lky"][k])
                        nc.vector._custom_dve(Q1, out=c0, in0=pxs, in1=rn2x[:],
                                              s0=c["wl"][k], s1=0.5)
                        nc.vector._custom_dve(Q2, out=c0, in0=c0, in1=rp2x[:],
                                              s0=c["wr"][k], s1=0.5)
                        nc.vector._custom_dve(Q1, out=c1, in0=pys, in1=rn2y[:],
                                              s0=c["wt"][k], s1=0.5)
                        nc.vector._custom_dve(Q2, out=c1, in0=c1, in1=rp2y[:],
                                              s0=c["wb"][k], s1=0.5)
                    if c["isl"][k]:
                        ch = nt[:, :, 20 + 2 * k]
                        sh = nt[:, :, 21 + 2 * k]
                        nc.vector._custom_dve(LIN, out=tout[:, :, 6 * k + 2], in0=ch,
                                              in1=sh, s0=c["scos"][k], s1=-c["ssin"][k])
                        nc.vector._custom_dve(LIN, out=tout[:, :, 6 * k + 3], in0=ch,
                                              in1=sh, s0=c["ssin"][k], s1=c["scos"][k])
                        nc.vector._custom_dve(LIN, out=tout[:, :, 6 * k + 4], in0=c0,
                                              in1=ch, s0=1.0, s1=c["slnx"][k])
                        nc.vector._custom_dve(LIN, out=tout[:, :, 6 * k + 5], in0=c1,
                                              in1=sh, s0=1.0, s1=c["slny"][k])
                    else:
                        jj = 20 + 2 * k if c["rr"][k] else 2 * k
                        nc.vector.tensor_copy(tout[:, :, 6 * k + 2:6 * k + 4],
                                              nt[:, :, jj:jj + 2])
                        nc.gpsimd.tensor_copy(tout[:, :, 6 * k + 4:6 * k + 6],
                                              tout[:, :, 6 * k:6 * k + 2])
                    pxs, pys = c0, c1
                nc.sync.dma_start(outv[:, off:off + F, :], tout[:])
                off += F
    nc.compile()
    return nc


def kernel(**inputs):
    var = np.ascontiguousarray(np.asarray(inputs["var_tensor"], dtype=np.float32))
    B = var.shape[0]
    assert B % (N_CORES * P) == 0
    b_core = B // N_CORES
    c = _host_consts(
        np.asarray(inputs["slider_lengths"]), np.asarray(inputs["slider_cos_each"]),
        np.asarray(inputs["slider_sin_each"]), np.asarray(inputs["note_distances"]),
        np.asarray(inputs["tick_diff"]), np.asarray(inputs["start_pos"]),
        np.asarray(inputs["is_slider"]))
    key = (B, tuple(sorted(c.items())))
    if key not in _NC_CACHE:
        _NC_CACHE[key] = _build(c, b_core)
    nc = _NC_CACHE[key]

    from concourse.bass_utils import run_bass_kernel_spmd
    in_maps = [{"var": var[i * b_core:(i + 1) * b_core]} for i in range(N_CORES)]
    res = run_bass_kernel_spmd(nc, in_maps, core_ids=list(range(N_CORES)))
    out = np.concatenate([r["out"] for r in res.results], axis=0)
    return out.reshape(B, NGS, 6)


# revision 14
# speedup vs baseline: 1.0319x; 1.0164x over previous
"""Trainium2 Bass kernel for nn_KerasCustomMappingLayer (osu-style map construction).

Strategy (pure data-parallel over 8 NeuronCores, B=1048576 rows):
  - Each core handles B/8 = 131072 rows laid out as 128 partitions x 1024
    elements; processed in ~7 tiles of F~147 elements per partition.
  - All 10 per-step scalars are host-known at build time -> each scan step is
    specialized on (rerand, is_slider). Positions are kept in the scaled
    domain x/XMAX, y/YMAX so outputs c0/c1/c4/c5 need no extra division.
  - rsqrt(c^2+s^2) via custom DVE hypot2 + ACT Exp(-0.5*Ln(x)) (Rsqrt banned).
  - The wall-clamp update is ONE custom DVE op per axis:
      _x = px + dx + 2*((px<wl)*relu(-dx) - (px>wr)*relu(dx))
  - Slider outputs via a lincomb custom op: out = a*C0 + b*C1.
  - Circle outputs are ACT copies; engine balance: DVE customs + 2 NT muls,
    ACT ln/exp/dx/copies, GPSIMD 2 NT muls.
"""
import sys
import numpy as np

for _p in ("/opt/trn_rl_repo",):
    if _p not in sys.path:
        sys.path.insert(0, _p)

NGS = 10
XMAX, YMAX = 512.0, 384.0
LMUL, MTFD = 1.0, 1.0
N_CORES = 8
P = 128

_OPS = {}
_NC_CACHE = {}


def _get_custom_ops():
    global _OPS
    if _OPS:
        return _OPS
    import concourse.dve_ops as dve_ops
    from concourse.dve_spec import Spec, Src0, Src1, C0, C1, C2, relu, sq
    from concourse.dve_uop import DveOpSpec

    defs = {
        "ANT_HYPOT2": dict(
            body=sq(Src0) + sq(Src1),
            reference=lambda in0, in1, s0, s1, imm2: (
                in0.astype(np.float32) ** 2 + in1.astype(np.float32) ** 2),
        ),
        # t1 = px + (px<wl)*RN2 - 0.5*RN2        (RN2 = relu(-2dx))
        "ANT_WALLQ1": dict(
            body=Src0 + (Src0 < C0) * Src1 - Src1 * C1,
            reference=lambda in0, in1, s0, s1, imm2: (
                in0.astype(np.float32) + (in0 < s0) * in1 - in1 * np.float32(s1)),
        ),
        # _x = t1 + 0.5*RP2 - (t1>wr)*RP2        (RP2 = relu(+2dx))
        "ANT_WALLQ2": dict(
            body=Src0 + Src1 * C1 - (Src0 > C0) * Src1,
            reference=lambda in0, in1, s0, s1, imm2: (
                in0.astype(np.float32) + in1 * np.float32(s1) - (in0 > s0) * in1),
        ),
        "ANT_LINCOMB": dict(
            body=Src0 * C0 + Src1 * C1,
            reference=lambda in0, in1, s0, s1, imm2: (
                in0.astype(np.float32) * s0 + in1.astype(np.float32) * s1),
        ),
    }
    ops = {}
    for name, d in defs.items():
        existing = next((o for o in dve_ops.OPS if o.name == name), None)
        if existing is not None:
            ops[name] = existing
            continue
        spec = Spec(body=d["body"], reference=d["reference"])
        row = max(dve_ops._SUB_OPCODE_FOR_NAME.values()) + 1
        assert row < 0x20, "custom DVE row overflow"
        dve_ops._SUB_OPCODE_FOR_NAME[name] = row
        shas = {}
        for ver in ("v3", "v4"):
            try:
                uops = dve_ops.lower(spec, ver=ver)
                shas[ver] = DveOpSpec(
                    name=name, opcode=row, uops=uops,
                    rd1_en=dve_ops.has_src1(spec)).sha(ver)
            except Exception:
                pass
        assert shas, f"lower() failed for {name}"
        op = dve_ops.DveOp(name, spec, subdim=False, uops_sha=shas)
        dve_ops.OPS.append(op)
        dve_ops.CUSTOM_DVE_SPECS[name] = spec
        ops[name] = op
    _OPS = ops
    return ops


def _host_consts(slider_lengths, slider_cos_each, slider_sin_each,
                 note_distances, tick_diff, start_pos, is_slider):
    f = np.float32
    l = (f(LMUL) * note_distances.astype(f)).astype(f)
    return dict(
        wl=tuple(float(x) for x in (f(0.05 * XMAX) + l * f(0.5)) / f(XMAX)),
        wr=tuple(float(x) for x in (f(0.95 * XMAX) - l * f(0.5)) / f(XMAX)),
        wt=tuple(float(x) for x in (f(0.05 * YMAX) + l * f(0.5)) / f(YMAX)),
        wb=tuple(float(x) for x in (f(0.95 * YMAX) - l * f(0.5)) / f(YMAX)),
        lkx=tuple(float(x) for x in l / f(XMAX)),
        lky=tuple(float(x) for x in l / f(YMAX)),
        rr=tuple(int(x) for x in (tick_diff.astype(f) > f(MTFD))),
        isl=tuple(int(x) for x in (np.asarray(is_slider) != 0)),
        slnx=tuple(float(x) for x in slider_lengths.astype(f) / f(XMAX)),
        slny=tuple(float(x) for x in slider_lengths.astype(f) / f(YMAX)),
        scos=tuple(float(x) for x in slider_cos_each.astype(f)),
        ssin=tuple(float(x) for x in slider_sin_each.astype(f)),
        px0=float(f(start_pos[0]) / f(XMAX)),
        py0=float(f(start_pos[1]) / f(YMAX)),
    )


def _build(c, b_core, n_tiles=8):
    import concourse.bacc as bacc
    import concourse.mybir as mybir
    from concourse.tile import TileContext

    f32 = mybir.dt.float32
    AF = mybir.ActivationFunctionType
    ops = _get_custom_ops()
    HYP, LIN = ops["ANT_HYPOT2"], ops["ANT_LINCOMB"]
    Q1, Q2 = ops["ANT_WALLQ1"], ops["ANT_WALLQ2"]

    npp = b_core // P                       # elements per partition (1024)
    base, rem = divmod(npp, n_tiles)
    Fs = [base + (1 if t < rem else 0) for t in range(n_tiles)]

    # which normalized pairs j are consumed, given the specialized steps:
    #   wall step k (rr=0): NT pair k;  circle rr=0: NT pair k (covered)
    #   slider k or circle rr=1: NTH pair 10+k
    needed = {k for k in range(NGS) if not c["rr"][k]}
    needed |= {NGS + k for k in range(NGS) if c["isl"][k] or c["rr"][k]}
    j0, j1 = min(needed), max(needed) + 1          # contiguous cover window
    njl = max(0, min(j1, NGS) - j0)                # low-half pairs in window
    njh = max(0, j1 - max(j0, NGS))                # high-half pairs in window

    nc = bacc.Bacc("TRN2", target_bir_lowering=False, debug=False)
    var = nc.dram_tensor("var", [b_core, 2 * NGS * 2], f32, kind="ExternalInput")
    out = nc.dram_tensor("out", [b_core, NGS * 6], f32, kind="ExternalOutput")
    varv = var.rearrange("(p n) c -> p n c", p=P)
    outv = out.rearrange("(p n) c -> p n c", p=P)

    with TileContext(nc) as tc:
        with tc.tile_pool(name="io", bufs=2) as iop, \
             tc.tile_pool(name="work", bufs=2) as wp, \
             tc.tile_pool(name="ph0", bufs=1) as ph0p, \
             tc.tile_pool(name="cst", bufs=1) as cp:
            Fmax = max(Fs)
            px0t = cp.tile([P, Fmax], f32, tag="px0")
            py0t = cp.tile([P, Fmax], f32, tag="py0")
            nc.vector.memset(px0t[:], c["px0"])
            nc.vector.memset(py0t[:], c["py0"])
            czero = cp.tile([P, 1], f32, tag="czero")
            chalf = cp.tile([P, 1], f32, tag="chalf")
            nc.vector.memset(czero[:], 0.0)
            nc.vector.memset(chalf[:], 0.5)
            nc.const_aps.aps[(f32, 0.0)] = czero[:]
            nc.const_aps.aps[(f32, 0.5)] = chalf[:]
            off = 0
            for F in Fs:
                tin = iop.tile([P, F, 4 * NGS], f32, tag="tin")
                nc.sync.dma_start(tin[:], varv[:, off:off + F, :])
                tout = iop.tile([P, F, 6 * NGS], f32, tag="tout")
                nt = wp.tile([P, F, 4 * NGS], f32, tag="nt")
                nj = j1 - j0
                ssum = ph0p.tile([P, F, nj], f32, tag="ssum")
                rn = ph0p.tile([P, F, nj], f32, tag="rn")

                # ---- phase 0: rn = (c^2+s^2)^-0.5 over the needed j window ----
                nc.vector._custom_dve(HYP, out=ssum[:], in0=tin[:, :, j0:j1],
                                      in1=tin[:, :, 20 + j0:20 + j1])
                nc.scalar.activation(rn[:], ssum[:], AF.Ln)
                nc.scalar.activation(rn[:], rn[:], AF.Exp, scale=-0.5)
                # nt per element: [c0 s0 c1 s1 .. c9 s9 | ch0 sh0 .. ch9 sh9]
                if njl:
                    lo0, lo1 = j0, j0 + njl
                    nc.vector.tensor_mul(nt[:, :, 2 * lo0:2 * lo1:2],
                                         tin[:, :, lo0:lo1], rn[:, :, 0:njl])
                    nc.gpsimd.tensor_mul(nt[:, :, 2 * lo0 + 1:2 * lo1:2],
                                         tin[:, :, 20 + lo0:20 + lo1], rn[:, :, 0:njl])
                if njh:
                    hi0 = max(j0, NGS)
                    nc.vector.tensor_mul(nt[:, :, 2 * hi0:2 * j1:2],
                                         tin[:, :, hi0:j1], rn[:, :, nj - njh:nj])
                    nc.gpsimd.tensor_mul(nt[:, :, 2 * hi0 + 1:2 * j1:2],
                                         tin[:, :, 20 + hi0:20 + j1], rn[:, :, nj - njh:nj])

                pxs, pys = px0t[:, 0:F], py0t[:, 0:F]
                for k in range(NGS):
                    c0 = tout[:, :, 6 * k]
                    c1 = tout[:, :, 6 * k + 1]
                    if c["rr"][k]:
                        # _x' = 0.5*vk + 0.5 ; _y' = 0.5*vk2 + 0.5  (one op, pair AP)
                        nc.vector.tensor_scalar(tout[:, :, 6 * k:6 * k + 2],
                                                tin[:, :, k:k + 21:20],
                                                0.5, 0.5,
                                                mybir.AluOpType.mult,
                                                mybir.AluOpType.add)
                    else:
                        rn2x = wp.tile([P, F], f32, tag="rn2x")
                        rp2x = wp.tile([P, F], f32, tag="rp2x")
                        rn2y = wp.tile([P, F], f32, tag="rn2y")
                        rp2y = wp.tile([P, F], f32, tag="rp2y")
                        nc.scalar.activation(rn2x[:], nt[:, :, 2 * k], AF.Relu,
                                             scale=-2.0 * c["lkx"][k])
                        nc.scalar.activation(rp2x[:], nt[:, :, 2 * k], AF.Relu,
                                             scale=2.0 * c["lkx"][k])
                        nc.scalar.activation(rn2y[:], nt[:, :, 2 * k + 1], AF.Relu,
                                             scale=-2.0 * c["lky"][k])
                        nc.scalar.activation(rp2y[:], nt[:, :, 2 * k + 1], AF.Relu,
                                             scale=2.0 * c["lky"][k])
                        nc.vector._custom_dve(Q1, out=c0, in0=pxs, in1=rn2x[:],
                                              s0=c["wl"][k], s1=0.5)
                        nc.vector._custom_dve(Q2, out=c0, in0=c0, in1=rp2x[:],
                                              s0=c["wr"][k], s1=0.5)
                        nc.vector._custom_dve(Q1, out=c1, in0=pys, in1=rn2y[:],
                                              s0=c["wt"][k], s1=0.5)
                        nc.vector._custom_dve(Q2, out=c1, in0=c1, in1=rp2y[:],
                                              s0=c["wb"][k], s1=0.5)
                    if c["isl"][k]:
                        ch = nt[:, :, 20 + 2 * k]
                        sh = nt[:, :, 21 + 2 * k]
                        nc.vector._custom_dve(LIN, out=tout[:, :, 6 * k + 2], in0=ch,
                                              in1=sh, s0=c["scos"][k], s1=-c["ssin"][k])
                        nc.vector._custom_dve(LIN, out=tout[:, :, 6 * k + 3], in0=ch,
                                              in1=sh, s0=c["ssin"][k], s1=c["scos"][k])
                        nc.vector._custom_dve(LIN, out=tout[:, :, 6 * k + 4], in0=c0,
                                              in1=ch, s0=1.0, s1=c["slnx"][k])
                        nc.vector._custom_dve(LIN, out=tout[:, :, 6 * k + 5], in0=c1,
                                              in1=sh, s0=1.0, s1=c["slny"][k])
                    else:
                        jj = 20 + 2 * k if c["rr"][k] else 2 * k
                        nc.vector.tensor_copy(tout[:, :, 6 * k + 2:6 * k + 4],
                                              nt[:, :, jj:jj + 2])
                        nc.gpsimd.tensor_copy(tout[:, :, 6 * k + 4:6 * k + 6],
                                              tout[:, :, 6 * k:6 * k + 2])
                    pxs, pys = c0, c1
                nc.sync.dma_start(outv[:, off:off + F, :], tout[:])
                off += F
    nc.compile()
    return nc


def kernel(**inputs):
    var = np.ascontiguousarray(np.asarray(inputs["var_tensor"], dtype=np.float32))
    B = var.shape[0]
    assert B % (N_CORES * P) == 0
    b_core = B // N_CORES
    c = _host_consts(
        np.asarray(inputs["slider_lengths"]), np.asarray(inputs["slider_cos_each"]),
        np.asarray(inputs["slider_sin_each"]), np.asarray(inputs["note_distances"]),
        np.asarray(inputs["tick_diff"]), np.asarray(inputs["start_pos"]),
        np.asarray(inputs["is_slider"]))
    key = (B, tuple(sorted(c.items())))
    if key not in _NC_CACHE:
        _NC_CACHE[key] = _build(c, b_core)
    nc = _NC_CACHE[key]

    from concourse.bass_utils import run_bass_kernel_spmd
    in_maps = [{"var": var[i * b_core:(i + 1) * b_core]} for i in range(N_CORES)]
    res = run_bass_kernel_spmd(nc, in_maps, core_ids=list(range(N_CORES)))
    out = np.concatenate([r["out"] for r in res.results], axis=0)
    return out.reshape(B, NGS, 6)
